# revision 1
# baseline (speedup 1.0000x reference)
"""GNN message passing (SAGEConv mean-agg + MLP + cdist) on 8 Trainium2 NeuronCores.

Sharding: nodes split 1024/core by destination; edges partitioned by dst block.
Aggregation on-device via dma_gather(x[src]) + one-hot matmul into PSUM.
cdist via K=4 matmul ([-2y; ones] @ [y_full; sq_full]) + sqrt epilogue.

Self-contained: hardcodes all shapes. kernel(**inputs) -> np.ndarray [8192, 8192].
"""
import numpy as np

import concourse.bacc as bacc
import concourse.mybir as mybir
import concourse.tile as tile
from concourse.bass_utils import run_bass_kernel_spmd
from concourse.masks import make_identity

N = 8192
E = 262144
D = 512
NCORE = 8
NLOC = N // NCORE      # 1024 nodes per core
NSB = NLOC // 128      # 8 subblocks of 128 dsts per core
P = 128

# numeric config
GATHER_BF16 = True     # False: gather fp32 rows as float32r; True: gather fp16 rows
CDIST_F32R = True      # False: cdist matmul in fp32 (safest); True: float32r
TRIANGLE = True        # True: compute upper-triangle superset, host mirrors rest


def _build(cap, n2=0, n3=0):
    """Build the SPMD kernel. cap = padded slot capacity per (core, subblock),
    multiple of 256. n2/n3 = static chunk counts for 2nd/3rd-destination
    one-hot passes (slots with multiple edges are ordered first)."""
    import os as _os
    phases = int(_os.environ.get("KERNEL_PHASES", "4"))
    repeat = int(_os.environ.get("KERNEL_REPEAT", "1"))
    dt = mybir.dt
    caph = cap // 2            # edges per gather slab (2 slabs per subblock)
    sh = caph // 16            # idx columns per slab
    nch = caph // 128          # chunks per slab
    g_dt = dt.float16 if GATHER_BF16 else dt.float32r
    c_dt = dt.float32r if CDIST_F32R else dt.float32  # cdist matmul operand dtype
    o_dt = dt.float16  # output written fp16, upcast to fp32 on host

    nq = int(_os.environ.get("KERNEL_NQ", "4"))
    nc = bacc.Bacc("TRN2", target_bir_lowering=False, debug=False, num_devices=NCORE,
                   num_swdge_queues=nq)

    # ---- inputs (same shapes on every core; values differ per core) ----
    x_g = nc.dram_tensor("x_g", [N, D], g_dt, kind="ExternalInput").ap()
    xlocT = nc.dram_tensor("xlocT", [D, NLOC], g_dt, kind="ExternalInput").ap()
    idx = nc.dram_tensor("idx", [P, NSB * 2 * sh], dt.int16, kind="ExternalInput").ap()
    dstloc = nc.dram_tensor("dstloc", [P, NSB * 2 * nch], dt.float32, kind="ExternalInput").ap()
    if n2:
        dst2 = nc.dram_tensor("dst2", [P, NSB * n2], dt.float32, kind="ExternalInput").ap()
    if n3:
        dst3 = nc.dram_tensor("dst3", [P, NSB * n3], dt.float32, kind="ExternalInput").ap()
    iota = nc.dram_tensor("iota", [P, P], dt.float32, kind="ExternalInput").ap()
    w_l = nc.dram_tensor("w_l", [D, D], dt.float32r, kind="ExternalInput").ap()
    w_r = nc.dram_tensor("w_r", [D, D], g_dt, kind="ExternalInput").ap()
    w_a = nc.dram_tensor("w_a", [D, 256], dt.float32r, kind="ExternalInput").ap()
    w_1 = nc.dram_tensor("w_1", [256, P], dt.float32r, kind="ExternalInput").ap()
    w_2 = nc.dram_tensor("w_2", [P, 64], dt.float32r, kind="ExternalInput").ap()
    w_3 = nc.dram_tensor("w_3", [64, 3], dt.float32r, kind="ExternalInput").ap()
    b_l = nc.dram_tensor("b_l", [P, 4], dt.float32, kind="ExternalInput").ap()
    b_a = nc.dram_tensor("b_a", [P, 2], dt.float32, kind="ExternalInput").ap()
    b_1 = nc.dram_tensor("b_1", [P, 1], dt.float32, kind="ExternalInput").ap()
    b_2 = nc.dram_tensor("b_2", [64, 1], dt.float32, kind="ExternalInput").ap()
    b_3 = nc.dram_tensor("b_3", [3, 1], dt.float32, kind="ExternalInput").ap()
    deg = nc.dram_tensor("deg", [P, NSB], dt.float32, kind="ExternalInput").ap()
    onesrow = nc.dram_tensor("onesrow", [1, NLOC], c_dt, kind="ExternalInput").ap()

    out = nc.dram_tensor("out", [NLOC, N], o_dt, kind="ExternalOutput").ap()


    def _emit(tc):
        with (
            tc.tile_pool(name="ylong", bufs=1) as yl_pool,
            tc.tile_pool(name="dram", bufs=1, space="DRAM") as dr,
            tc.tile_pool(name="pw", bufs=1) as pw,
        ):
          with tc.tile_pool(name="pa", bufs=1) as pa:
            # ---- preload constants ----
            idx_t = pa.tile([P, NSB * 2, sh], dt.int16)
            nc.sync.dma_start(idx_t[:], idx.rearrange("p (b s) -> p b s", s=sh))
            dl_t = pa.tile([P, NSB * 2, nch], dt.float32)
            nc.sync.dma_start(dl_t[:], dstloc.rearrange("p (b c) -> p b c", c=nch))
            if n2:
                dl2_t = pa.tile([P, NSB, n2], dt.float32)
                nc.sync.dma_start(dl2_t[:], dst2.rearrange("p (b c) -> p b c", c=n2))
            if n3:
                dl3_t = pa.tile([P, NSB, n3], dt.float32)
                nc.sync.dma_start(dl3_t[:], dst3.rearrange("p (b c) -> p b c", c=n3))
            iota_t = pa.tile([P, P], dt.float32)
            nc.sync.dma_start(iota_t[:], iota)
            ident_t = pa.tile([P, P], dt.float32)
            make_identity(nc, ident_t[:])
            deg_t = pa.tile([P, NSB], dt.float32)
            nc.sync.dma_start(deg_t[:], deg)
            ones3_t = pw.tile([3, 1], dt.float32)
            nc.vector.memset(ones3_t[:], 1.0)

            wl_t = pw.tile([P, 4, D], dt.float32r)
            nc.sync.dma_start(wl_t[:], w_l.rearrange("(k p) f -> p k f", p=P))
            wr_t = pw.tile([P, 4, D], g_dt)
            nc.sync.dma_start(wr_t[:], w_r.rearrange("(k p) f -> p k f", p=P))
            wa_t = pw.tile([P, 4, 256], dt.float32r)
            nc.sync.dma_start(wa_t[:], w_a.rearrange("(k p) f -> p k f", p=P))
            w1_t = pw.tile([P, 2, P], dt.float32r)
            nc.sync.dma_start(w1_t[:], w_1.rearrange("(k p) f -> p k f", p=P))
            w2_t = pw.tile([P, 64], dt.float32r)
            nc.sync.dma_start(w2_t[:], w_2)
            w3_t = pw.tile([64, 3], dt.float32r)
            nc.sync.dma_start(w3_t[:], w_3)
            bl_t = pw.tile([P, 4], dt.float32)
            nc.sync.dma_start(bl_t[:], b_l)
            ba_t = pw.tile([P, 2], dt.float32)
            nc.sync.dma_start(ba_t[:], b_a)
            b1_t = pw.tile([P, 1], dt.float32)
            nc.sync.dma_start(b1_t[:], b_1)
            b2_t = pw.tile([64, 1], dt.float32)
            nc.sync.dma_start(b2_t[:], b_2)
            b3_t = pw.tile([3, 1], dt.float32)
            nc.sync.dma_start(b3_t[:], b_3)

            xlT_t = pw.tile([P, 4, NLOC], g_dt)
            nc.sync.dma_start(xlT_t[:], xlocT.rearrange("(k p) n -> p k n", p=P))

            # aggT survives into phase 2
            aggT = pw.tile([P, 4, NLOC], dt.float32r)

            # ===== Phase 1 (aggregation) interleaved with phase 2 (MLP) =====
            # agg blocks 0-3 -> MLP half 0 -> AllGather(half 0) runs under
            # agg blocks 4-7 -> MLP half 1 -> AllGather(half 1): both
            # collectives hide beneath compute.
            ym = yl_pool.tile([5, NLOC], c_dt)
            sq_row = yl_pool.tile([1, NLOC], c_dt)
            yf = yl_pool.tile([5, NSB, NCORE, P], c_dt)
            H = NLOC // 2
            with (
                tc.tile_pool(name="p1", bufs=3) as p1,
                tc.tile_pool(name="p1ps", bufs=2, space="PSUM") as p1ps,
                tc.tile_pool(name="p2", bufs=1) as p2,
                tc.tile_pool(name="p2ps", bufs=2, space="PSUM") as p2ps,
                tc.tile_pool(name="p2ps1", bufs=1, space="PSUM") as p2ps1,
            ):
                sp = bool(int(_os.environ.get("KERNEL_SP", "0")))

                def agg_block(b):
                    if phases >= 1:
                        msg_ps = p1ps.tile([P, D], dt.float32, space="PSUM", tag="msg")
                    for s in range(2):
                        sl = b * 2 + s
                        xg = p1.tile([P, nch, D], g_dt, tag="xg")
                        nc.gpsimd.dma_gather(
                            xg[:], x_g, idx_t[:, sl, :],
                            num_idxs=caph, num_idxs_reg=caph, elem_size=D,
                            single_packet=sp, queue_num=sl % nq,
                        )
                        if phases < 1:
                            continue
                        s_all = p1.tile([P, nch, P], g_dt, tag="s")
                        nc.vector.tensor_tensor(
                            out=s_all[:],
                            in0=iota_t[:].rearrange("p (c d) -> p c d", c=1).to_broadcast([P, nch, P]),
                            in1=dl_t[:, sl, :].to_broadcast([P, nch, P]),
                            op=mybir.AluOpType.is_equal,
                        )
                        for c in range(nch):
                            nc.tensor.matmul(
                                msg_ps[:], lhsT=s_all[:, c, :], rhs=xg[:, c, :],
                                start=(s == 0 and c == 0), stop=(s == 1 and c == nch - 1),
                            )
                        if s == 0 and n2:
                            # 2nd/3rd-destination passes over slab-0 chunks
                            s2 = p1.tile([P, n2, P], g_dt, tag="s2")
                            nc.vector.tensor_tensor(
                                out=s2[:],
                                in0=iota_t[:].rearrange("p (c d) -> p c d", c=1).to_broadcast([P, n2, P]),
                                in1=dl2_t[:, b, :].to_broadcast([P, n2, P]),
                                op=mybir.AluOpType.is_equal,
                            )
                            for c in range(n2):
                                nc.tensor.matmul(
                                    msg_ps[:], lhsT=s2[:, c, :], rhs=xg[:, c, :],
                                    start=False, stop=False,
                                )
                        if s == 0 and n3:
                            s3 = p1.tile([P, n3, P], g_dt, tag="s3")
                            nc.vector.tensor_tensor(
                                out=s3[:],
                                in0=iota_t[:].rearrange("p (c d) -> p c d", c=1).to_broadcast([P, n3, P]),
                                in1=dl3_t[:, b, :].to_broadcast([P, n3, P]),
                                op=mybir.AluOpType.is_equal,
                            )
                            for c in range(n3):
                                nc.tensor.matmul(
                                    msg_ps[:], lhsT=s3[:, c, :], rhs=xg[:, c, :],
                                    start=False, stop=False,
                                )
                    if phases < 1:
                        return
                    degm = p1.tile([P, 1], dt.float32, tag="degm")
                    nc.vector.tensor_scalar_max(degm[:], deg_t[:, b:b + 1], 1.0)
                    rec = p1.tile([P, 1], dt.float32, tag="rec")
                    nc.vector.reciprocal(rec[:], degm[:])
                    agg_b = p1.tile([P, D], dt.float32, tag="agg")
                    nc.vector.tensor_scalar_mul(agg_b[:], msg_ps[:], rec[:, :1])
                    # transpose agg_b -> aggT[:, k, b*128:(b+1)*128]
                    for k in range(4):
                        tr_ps = p1ps.tile([P, P], dt.float32, space="PSUM", tag="tr")
                        nc.tensor.transpose(tr_ps[:], agg_b[:, k * P:(k + 1) * P], ident_t[:])
                        nc.vector.tensor_copy(aggT[:, k, b * P:(b + 1) * P], tr_ps[:])

                # ---- phase-2 tiles (MLP is emitted per node-half below) ----
                h1 = p2.tile([P, 4, NLOC], dt.float32r)
                h2 = p2.tile([P, 2, NLOC], dt.float32r)
                h3 = p2.tile([P, NLOC], dt.float32r)
                h4 = p2.tile([64, NLOC], dt.float32r)
                yr = p2.tile([3, NLOC], c_dt)
                y2 = p2.tile([3, NLOC], dt.float32)
                yr_f = yr[:].bitcast(dt.float32) if CDIST_F32R else yr[:]
                nc.sync.dma_start(ym[3:4, :], onesrow)

                def mlp_half(hh):
                    hs = slice(hh * D, (hh + 1) * D)
                    for o in range(4):
                        ps = p2ps.tile([P, D], dt.float32, space="PSUM", tag="h")
                        for k in range(4):
                            nc.tensor.matmul(
                                ps[:], lhsT=wl_t[:, k, o * P:(o + 1) * P],
                                rhs=aggT[:, k, hs],
                                start=(k == 0), stop=False,
                            )
                        for k in range(4):
                            nc.tensor.matmul(
                                ps[:], lhsT=wr_t[:, k, o * P:(o + 1) * P],
                                rhs=xlT_t[:, k, hs],
                                start=False, stop=(k == 3),
                            )
                        nc.scalar.activation(
                            h1[:, o, hs], ps[:],
                            mybir.ActivationFunctionType.Relu, bias=bl_t[:, o:o + 1],
                        )
                    for o in range(2):
                        ps = p2ps.tile([P, D], dt.float32, space="PSUM", tag="h")
                        for k in range(4):
                            nc.tensor.matmul(
                                ps[:], lhsT=wa_t[:, k, o * P:(o + 1) * P],
                                rhs=h1[:, k, hs],
                                start=(k == 0), stop=(k == 3),
                            )
                        nc.scalar.activation(
                            h2[:, o, hs], ps[:],
                            mybir.ActivationFunctionType.Relu, bias=ba_t[:, o:o + 1],
                        )
                    ps = p2ps.tile([P, D], dt.float32, space="PSUM", tag="h")
                    for k in range(2):
                        nc.tensor.matmul(
                            ps[:], lhsT=w1_t[:, k, :], rhs=h2[:, k, hs],
                            start=(k == 0), stop=(k == 1),
                        )
                    nc.scalar.activation(
                        h3[:, hs], ps[:],
                        mybir.ActivationFunctionType.Relu, bias=b1_t[:, :1],
                    )
                    ps4 = p2ps.tile([64, D], dt.float32, space="PSUM", tag="h")
                    nc.tensor.matmul(ps4[:], lhsT=w2_t[:], rhs=h3[:, hs],
                                     start=True, stop=True)
                    nc.scalar.activation(
                        h4[:, hs], ps4[:],
                        mybir.ActivationFunctionType.Relu, bias=b2_t[:, :1],
                    )
                    psy = p2ps1.tile([3, D], dt.float32, space="PSUM", tag="y")
                    nc.tensor.matmul(psy[:], lhsT=w3_t[:], rhs=h4[:, hs],
                                     start=True, stop=True)
                    nc.scalar.activation(
                        yr[:, hs], psy[:],
                        mybir.ActivationFunctionType.Identity, bias=b3_t[:, :1],
                    )
                    # y2 = yr*yr (fp32); |y|^2 row via tiny matmul
                    nc.vector.tensor_tensor(out=y2[:, hs], in0=yr_f[:, hs],
                                            in1=yr_f[:, hs],
                                            op=mybir.AluOpType.mult)
                    nc.vector.tensor_scalar_mul(ym[0:3, hs], yr_f[:, hs], -2.0)
                    pss = p2ps1.tile([1, D], dt.float32, space="PSUM", tag="y")
                    nc.tensor.matmul(pss[:], lhsT=ones3_t[:, :1], rhs=y2[:, hs],
                                     start=True, stop=True)
                    nc.vector.tensor_copy(sq_row[:, hs], pss[:])
                    nc.sync.dma_start(ym[4:5, hs], sq_row[:, hs])
                    if phases >= 3:
                        # stage the AllGather payload; the collective itself
                        # is emitted later so it never blocks the Pool-queue
                        # gather stream.
                        ag_in = dr.tile([5, H], c_dt, tag=f"agi{hh}")
                        nc.sync.dma_start(ag_in[0:3, :], yr[:, hs])
                        nc.sync.dma_start(ag_in[3:4, :], sq_row[:, hs])
                        nc.sync.dma_start(ag_in[4:5, :], onesrow[:, 0:H])
                    return ag_in if phases >= 3 else None

                def ag_coll(hh, ag_in):
                    # AllGather [y(3); sq(1); ones(1)] for half hh = local
                    # nodes [hh*512,(hh+1)*512) = global column blocks
                    # [32hh, 32hh+32) in yf (row-blocks interleaved by core).
                    ag_out = dr.tile([NCORE, 5, H], c_dt, tag=f"ago{hh}")
                    nc.gpsimd.collective_compute(
                        "AllGather", mybir.AluOpType.bypass,
                        replica_groups=[list(range(NCORE))],
                        ins=[ag_in[:].opt()], outs=[ag_out[:].opt()],
                    )
                    return ag_out

                def yf_dma(hh, ag_out):
                    # split per column-block: bounds head-of-line blocking on
                    # the DMA queue and lets cdist start on the first chunk
                    ag_r = ag_out[:].rearrange("r p (t w) -> p t r w", w=P)
                    for tb in range(4):
                        nc.sync.dma_start(
                            yf[:, 4 * hh + tb, :, :], ag_r[:, tb, :, :])

                # waves: agg 0-3, MLP half 0, agg 4-7 (gathers stay dense on
                # the Pool queue), collective 0 under MLP half 1. The yf-A
                # reload is emitted after half-1's staging DMAs so, on the
                # in-order SP queue, collective 1's inputs aren't stuck
                # behind it.
                for b in range(4):
                    agg_block(b)
                if phases >= 2:
                    ag0 = mlp_half(0)
                for b in range(4, NSB):
                    agg_block(b)
                if phases >= 3:
                    ago0 = ag_coll(0, ag0)
                if phases >= 2:
                    ag1 = mlp_half(1)
                if phases >= 3:
                    yf_dma(0, ago0)
                    ago1 = ag_coll(1, ag1)
                    yf_dma(1, ago1)

          if phases == 1:
            # debug: dump aggT so phase-1 output is observable
            nc.sync.dma_start(out[0:P, :].bitcast(dt.float32),
                              aggT[:].bitcast(dt.float32).rearrange("p k n -> p (k n)"))
          if phases >= 4:
            # ================= Phase 4: cdist (upper-triangle superset) =====
            # Core c's row-block t covers global rows (8t+c)*128..; columns
            # j*512 >= t*1024 form a uniform-per-core superset of the upper
            # triangle. The host mirrors the rest from the transpose.
            # d2[i, j] = -2 y_i.y_j + sq_j + sq_i via one K=5 matmul; then
            # clamp to 0 on DVE and sqrt to fp16 on Activation.
            with (
                tc.tile_pool(name="p4", bufs=2) as p4,
                tc.tile_pool(name="p4ps", bufs=2, space="PSUM") as p4ps,
            ):
                yf_flat = yf[:].rearrange("p t r w -> p (t r w)")

                def cd_span(t, ja, jb):
                    """Emit cdist for row-block t, column tiles [ja, jb)."""
                    w = jb - ja
                    stage = p4.tile([P, 8, D], o_dt, tag="stage")
                    for g0 in range(0, w, 4):
                        gsz = min(4, w - g0)
                        # matmuls into adjacent PSUM banks, then one wide
                        # DVE clamp + one wide Act sqrt over all banks
                        ps = p4ps.tile([P, 4, D], dt.float32, space="PSUM",
                                       tag="cd")
                        for j4 in range(gsz):
                            j = ja + g0 + j4
                            nc.tensor.matmul(
                                ps[:, j4, :], lhsT=ym[:, t * P:(t + 1) * P],
                                rhs=yf_flat[:, j * D:(j + 1) * D],
                                start=True, stop=True,
                            )
                        d2t = p4.tile([P, 4, D], o_dt, tag="d2")
                        nc.vector.tensor_scalar_max(d2t[:, 0:gsz, :], ps[:, 0:gsz, :], 0.0)
                        nc.scalar.activation(stage[:, g0:g0 + gsz, :], d2t[:, 0:gsz, :],
                                             mybir.ActivationFunctionType.Sqrt)
                    nc.sync.dma_start(
                        out[t * P:(t + 1) * P, ja * D:jb * D],
                        stage[:, 0:w, :].rearrange("p a b -> p (a b)"),
                    )

                # pass 1: column tiles served by the half-0 collective
                # (j < 8) — these run while the half-1 collective is in
                # flight. pass 2: the rest.
                for t in range(NSB):
                    j0 = 2 * t if TRIANGLE else 0
                    if j0 < 8:
                        cd_span(t, j0, min(8, 16))
                for t in range(NSB):
                    j0 = max(8, 2 * t) if TRIANGLE else 8
                    if j0 < 16:
                        cd_span(t, j0, 16)

    with tile.TileContext(nc) as tc:
        for _rep in range(repeat):
            _emit(tc)

    nc.compile()
    return nc


def _prep_inputs(x, edge_index, W_l, b_l, W_r, Wa, ba, W1, b1, W2, b2, W3, b3):
    """Host-side sharding/layout. Returns ((cap, n2, n3), in_maps)."""
    src = np.asarray(edge_index[0], dtype=np.int64)
    dst = np.asarray(edge_index[1], dtype=np.int64)
    order = np.argsort(dst, kind="stable")
    src_s = src[order].astype(np.int32)
    dst_s = dst[order].astype(np.int32)
    bounds = np.searchsorted(dst_s, np.arange(0, N + 1, P))  # 65 boundaries

    # Per (global block): dedup srcs into slots. A slot carries up to 3
    # destinations (d1/d2/d3); extra edges of high-multiplicity srcs spill
    # into appended single slots. Slots with >=2 edges are ordered first so
    # the d2/d3 one-hot passes cover a compact chunk prefix.
    blk = []
    for g in range(N // P):
        lo, hi = bounds[g], bounds[g + 1]
        ss = src_s[lo:hi]
        sd = dst_s[lo:hi] - g * P
        o2 = np.argsort(ss, kind="stable")
        ss, sd = ss[o2], sd[o2]
        u, i0, cnt = np.unique(ss, return_index=True, return_counts=True)
        d1 = sd[i0]
        d2 = np.where(cnt >= 2, sd[np.minimum(i0 + 1, sd.size - 1)], -1)
        d3 = np.where(cnt >= 3, sd[np.minimum(i0 + 2, sd.size - 1)], -1)
        spill_mask = cnt > 3
        sp_src, sp_d = [], []
        for si in np.nonzero(spill_mask)[0]:
            for e in sd[i0[si] + 3:i0[si] + cnt[si]]:
                sp_src.append(u[si])
                sp_d.append(e)
        k = np.minimum(cnt, 3)
        perm = np.argsort(-k, kind="stable")  # 3s, then 2s, then singles
        slot_src = np.concatenate([u[perm], np.asarray(sp_src, np.int32)])
        sd1 = np.concatenate([d1[perm], np.asarray(sp_d, np.int32)])
        neg = np.full(len(sp_src), -1, np.int32)
        sd2 = np.concatenate([d2[perm], neg])
        sd3 = np.concatenate([d3[perm], neg])
        m2 = int((cnt >= 2).sum())
        m3 = int((cnt >= 3).sum())
        blk.append((slot_src, sd1, sd2, sd3, m2, m3))

    cap = max(b[0].size for b in blk)
    cap = ((cap + 255) // 256) * 256  # multiple of 256 (two equal slabs)
    n2 = (max(b[4] for b in blk) + P - 1) // P
    n3 = (max(b[5] for b in blk) + P - 1) // P
    caph = cap // 2
    sh = caph // 16
    nch = caph // 128

    x = np.asarray(x, dtype=np.float32)
    if GATHER_BF16:
        x_g = x.astype(np.float16)
    else:
        x_g = x
    iota = np.tile(np.arange(P, dtype=np.float32)[None, :], (P, 1))

    def bias_cols(b, parts):
        b = np.asarray(b, dtype=np.float32).reshape(-1)
        k = b.size // parts
        return b.reshape(k, parts).T.copy()  # [parts, k]

    g_np = np.float16 if GATHER_BF16 else np.float32
    common = dict(
        x_g=x_g,
        iota=iota,
        onesrow=np.ones((1, NLOC), np.float32),
        w_l=np.asarray(W_l, np.float32), w_r=np.asarray(W_r, g_np),
        w_a=np.asarray(Wa, np.float32), w_1=np.asarray(W1, np.float32),
        w_2=np.asarray(W2, np.float32), w_3=np.asarray(W3, np.float32),
        b_l=bias_cols(b_l, P), b_a=bias_cols(ba, P), b_1=bias_cols(b1, P),
        b_2=bias_cols(b2, 64), b_3=bias_cols(b3, 3),
    )

    in_maps = []
    for c in range(NCORE):
        idx_core = np.zeros((NSB * 2, caph), np.int16)
        dl_core = np.full((NSB * 2, caph), -1.0, np.float32)
        d2_core = np.full((NSB, n2 * P), -1.0, np.float32)
        d3_core = np.full((NSB, n3 * P), -1.0, np.float32)
        for b in range(NSB):
            g = b * NCORE + c  # interleaved: core c owns global blocks {8t+c}
            slot_src, sd1, sd2, sd3, m2, m3 = blk[g]
            cnt = slot_src.size
            buf_s = np.zeros(cap, np.int16)
            buf_d = np.full(cap, -1.0, np.float32)
            buf_s[:cnt] = slot_src.astype(np.int16)
            buf_d[:cnt] = sd1.astype(np.float32)
            idx_core[2 * b] = buf_s[:caph]
            idx_core[2 * b + 1] = buf_s[caph:]
            dl_core[2 * b] = buf_d[:caph]
            dl_core[2 * b + 1] = buf_d[caph:]
            d2_core[b, :m2] = sd2[:m2].astype(np.float32)
            d3_core[b, :m3] = sd3[:m3].astype(np.float32)
        # idx wrapped layout: [slab, 128, sh] with idx i at [i%16, i//16], rows tiled x8
        idx_w = idx_core.reshape(NSB * 2, sh, 16).transpose(0, 2, 1)  # [slab, 16, sh]
        idx_w = np.tile(idx_w, (1, 8, 1))  # [slab, 128, sh]
        idx_in = idx_w.transpose(1, 0, 2).reshape(P, NSB * 2 * sh).copy()
        # dstloc chunk-major: [slab, nch, 128] -> [128, slab, nch]
        dl_w = dl_core.reshape(NSB * 2, nch, P).transpose(2, 0, 1)
        dl_in = dl_w.reshape(P, NSB * 2 * nch).copy()
        d2_in = d2_core.reshape(NSB, n2, P).transpose(2, 0, 1).reshape(P, NSB * n2).copy()
        d3_in = d3_core.reshape(NSB, n3, P).transpose(2, 0, 1).reshape(P, NSB * n3).copy()

        deg_core = np.zeros((P, NSB), np.float32)
        for b in range(NSB):
            g = b * NCORE + c
            seg = dst_s[bounds[g]:bounds[g + 1]] - g * P
            deg_core[:, b] = np.bincount(seg, minlength=P).astype(np.float32)
        # local rows = global row-blocks {8t+c}, concatenated over t
        xloc = x.reshape(N // P, P, D)[c::NCORE].reshape(NLOC, D)
        m = dict(common)
        m["xlocT"] = np.ascontiguousarray(xloc.T).astype(g_np)
        m["deg"] = deg_core
        m["idx"] = idx_in
        m["dstloc"] = dl_in
        if n2:
            m["dst2"] = d2_in
        if n3:
            m["dst3"] = d3_in
        in_maps.append(m)
    return (cap, n2, n3), in_maps


_CACHED = {}


def kernel(**inputs):
    dims, in_maps = _prep_inputs(**inputs)
    key = (dims, GATHER_BF16, CDIST_F32R, TRIANGLE)
    if key not in _CACHED:
        _CACHED[key] = _build(*dims)
    nc = _CACHED[key]
    import os
    trace = bool(int(os.environ.get("KERNEL_TRACE", "0")))
    if trace:
        try:
            import antenv.axon_hooks  # noqa: F401
        except ImportError:
            import sys as _sys
            import types as _types
            _m = _types.ModuleType("antenv.axon_hooks")
            _m.get_axon_ntff_profile_hook = lambda: None
            _sys.modules["antenv.axon_hooks"] = _m
    res = run_bass_kernel_spmd(nc, in_maps, core_ids=list(range(NCORE)), trace=trace)
    kernel.last_results = res
    return _assemble([res.results[c]["out"] for c in range(NCORE)])


def _assemble(outs):
    """Interleave per-core row-blocks into the full [N, N] fp32 output and
    mirror the symmetric lower region the device skipped."""
    O = np.empty((N, N), np.float32)
    ob = O.reshape(N // P, P, N)
    for c in range(NCORE):
        oc = outs[c].astype(np.float32).reshape(NSB, P, N)
        for t in range(NSB):
            if TRIANGLE:
                ob[t * NCORE + c, :, t * NLOC:] = oc[t, :, t * NLOC:]
            else:
                ob[t * NCORE + c] = oc[t]
    if TRIANGLE:
        for rb in range(NCORE, N // P):  # row-blocks with t >= 1
            t = rb // NCORE
            O[rb * P:(rb + 1) * P, :t * NLOC] = O[:t * NLOC, rb * P:(rb + 1) * P].T
    return O

